# revision 5
# baseline (speedup 1.0000x reference)
"""CenterLoss kernel for Trainium2 (Bass/Tile), data-parallel over 8 NeuronCores.

reference:
    d_i = ||x_i||^2 + ||centers[l_i]||^2 - 2 x_i . centers[l_i]   (= ||x_i - c_{l_i}||^2)
    loss = mean_i clip(d_i, 1e-12, 1e12)

Only the label-gathered entry of the [N, C] distance matrix is used, so the
kernel never forms it: each core gathers centers[labels] with indirect DMA,
computes (x - c)^2 via DVE subtract + ACT square-with-accumulate, reduces to
a scalar partial sum, and the host combines the 8 partials into the mean.
The clip is a provable no-op for this input distribution (d_i ~ chi^2-like,
concentrated around 256; min over N is >> 1e-12).

Sharding: x/labels split into 8 contiguous row shards; centers replicated.
"""

import numpy as np

import concourse.bass as bass
import concourse.bacc as bacc
import concourse.tile as tile
from concourse import mybir
from concourse.bass_utils import run_bass_kernel_spmd

N, C, D = 65536, 1000, 128
N_CORES = 8
P = 128
ROWS_PER_CORE = N // N_CORES            # 8192
NBLK = ROWS_PER_CORE // P               # 64 row-blocks of 128 rows
CHUNK_BLKS = 16                         # row-blocks per compute chunk (1 MiB tiles)
NCHUNK = NBLK // CHUNK_BLKS             # 4

_NC = None


def _build_nc():
    f32 = mybir.dt.float32
    nc = bacc.Bacc(trn_type="TRN2")

    x = nc.dram_tensor("x", [ROWS_PER_CORE, D], f32, kind="ExternalInput")
    # labels pre-transposed on host to [P, NBLK]: labels[p, b] = label of row b*128+p
    labels = nc.dram_tensor("labels", [P, NBLK], mybir.dt.int32, kind="ExternalInput")
    centers = nc.dram_tensor("centers", [C, D], f32, kind="ExternalInput")
    out = nc.dram_tensor("out", [1, 1], f32, kind="ExternalOutput")

    # [NCHUNK, P, CHUNK_BLKS, D]; row (c*CHUNK_BLKS+q)*P + p -> chunk c, partition p, slot q
    x_r = x.ap().rearrange("(c q p) d -> c p q d", q=CHUNK_BLKS, p=P)

    with tile.TileContext(nc) as tc:
        with (
            tc.tile_pool(name="xp", bufs=2) as xp,
            tc.tile_pool(name="cp", bufs=2) as cp,
            tc.tile_pool(name="small", bufs=1) as small,
            tc.tile_pool(name="psp", bufs=1, space="PSUM") as psp,
        ):
            lab = small.tile([P, NBLK], mybir.dt.int32)
            nc.sync.dma_start(out=lab[:], in_=labels.ap())

            acc = small.tile([P, NCHUNK], f32)
            for c in range(NCHUNK):
                xt = xp.tile([P, CHUNK_BLKS * D], f32)
                nc.sync.dma_start(
                    out=xt[:].rearrange("p (q d) -> p q d", q=CHUNK_BLKS),
                    in_=x_r[c],
                )
                ct = cp.tile([P, CHUNK_BLKS * D], f32)
                # one gather for the whole chunk: out[p, q*D:(q+1)*D] = centers[lab[p, c*CHUNK_BLKS+q]]
                nc.gpsimd.indirect_dma_start(
                    out=ct[:],
                    out_offset=None,
                    in_=centers.ap(),
                    in_offset=bass.IndirectOffsetOnAxis(
                        ap=lab[:, c * CHUNK_BLKS:(c + 1) * CHUNK_BLKS], axis=0
                    ),
                )
                nc.vector.tensor_tensor(
                    out=xt[:], in0=xt[:], in1=ct[:], op=mybir.AluOpType.subtract
                )
                nc.scalar.activation(
                    out=xt[:],
                    in_=xt[:],
                    func=mybir.ActivationFunctionType.Square,
                    accum_out=acc[:, c:c + 1],
                )

            dsum = small.tile([P, 1], f32)
            nc.vector.tensor_reduce(
                out=dsum[:], in_=acc[:], axis=mybir.AxisListType.X,
                op=mybir.AluOpType.add,
            )
            ones = small.tile([P, 1], f32)
            nc.vector.memset(ones[:], 1.0)
            ps = psp.tile([1, 1], f32)
            nc.tensor.matmul(out=ps[:], lhsT=ones[:], rhs=dsum[:], start=True, stop=True)
            res = small.tile([1, 1], f32)
            nc.vector.tensor_copy(out=res[:], in_=ps[:])
            nc.sync.dma_start(out=out.ap(), in_=res[:])

    nc.compile()
    return nc


def _get_nc():
    global _NC
    if _NC is None:
        _NC = _build_nc()
    return _NC


def make_in_maps(x, labels, centers):
    x = np.ascontiguousarray(np.asarray(x), dtype=np.float32)
    labels_np = np.asarray(labels).astype(np.int32)
    centers = np.ascontiguousarray(np.asarray(centers), dtype=np.float32)
    in_maps = []
    for m in range(N_CORES):
        lo = m * ROWS_PER_CORE
        ls = np.ascontiguousarray(
            labels_np[lo:lo + ROWS_PER_CORE].reshape(NBLK, P).T
        )
        in_maps.append({
            "x": x[lo:lo + ROWS_PER_CORE],
            "labels": ls,
            "centers": centers,
        })
    return in_maps


def run(x, labels, centers, **spmd_kwargs):
    """Run on the 8 NeuronCores; returns (loss, BassKernelResults)."""
    nc = _get_nc()
    in_maps = make_in_maps(x, labels, centers)
    res = run_bass_kernel_spmd(nc, in_maps, core_ids=list(range(N_CORES)), **spmd_kwargs)
    total = sum(float(r["out"][0, 0]) for r in res.results)
    return np.float32(total / N), res


def kernel(x, labels, centers):
    loss, _ = run(x, labels, centers)
    return loss
